# revision 27
# baseline (speedup 1.0000x reference)
"""Backward-Euler 1D implicit diffusion step (tridiagonal solve) on 8 TRN2 cores.

Math: the system (I - dt*D*Lap) x = C has constant-coefficient tridiagonal
bands (-r, 1+2r, -r) away from the two Dirichlet rows, so away from the ends
the Thomas algorithm's elimination coefficients sit at their fixed point:
with s = sqrt((1+2r)^2 - 4r^2), mu = ((1+2r) - s)/(2r), delta = ((1+2r)+s)/2,
the solve is exactly
    v_i = mu*v_{i-1} + C_i      (forward first-order recurrence)
    y_i = mu*y_{i+1} + v_i      (backward first-order recurrence)
    x_i = y_i / delta
For r = 0.1, mu ~= 0.0839: the recurrences forget their initial state at
3e-10 relative within 8 elements, which makes the solve local — chunks can be
cut anywhere given an 8-element halo.  Both recurrences map 1:1 onto the DVE
``tensor_tensor_scan`` instruction ((mult, add) per-partition prefix scan;
the backward one runs on negative-stride APs), and the 1/delta scale is
folded into the host-side input prep, so the whole solve is 2 DVE scans per
tile.  The few boundary-influenced rows at each end (where the Thomas
coefficients have not yet converged to the fixed point and the Dirichlet rows
replace C) are fixed up on host with an exact float64 Thomas solve on a small
window.

Sharding: grid split into 8 contiguous shards, one per NeuronCore; halos are
materialized on host, so cores are fully independent (no collectives).
Per-core layout: partition p owns the contiguous sub-chunk
[p*8192, p*8192+8192) of the shard, stored with an 8-element halo per side as
a (128, 8209) DRAM tensor (col 0 carries mu — the scan's data0 operand must
live in SBUF, and folding it into tile 0's load keeps every instruction at
the <=1 semaphore wait this compiler build tolerates), processed as 8
pipelined tiles.  Loads ride SWDGE (gpsimd queue, otherwise
idle) so the HWDGE ring only carries stores — ring-slot waits would otherwise
stack a second wait onto the DMAs.  Tile sizes taper at both ends so the DVE
pipeline starts early and drains quickly."""

import os
import sys

import numpy as np

for _p in ("/opt/trn_rl_repo", "/root/.axon_site/_ro/trn_rl_repo"):
    if os.path.isdir(_p) and _p not in sys.path:
        sys.path.insert(0, _p)

NX = 8388608
NCORES = 8
P = 128
SHARD = NX // NCORES            # 1048576 per core
FPT = SHARD // P                # 8192 per partition
H = 8                           # halo (FIR half-width)
# per-core tile schedule: small head tile starts the DVE pipeline early,
# small tail tiles shorten the drain (store dispatch cost scales with width,
# so the last stores must be cheap); found by model search.  8 HWDGE DMAs max
# (the 8 DMA-HW ring procs each tolerate one without a ring-slot wait), so
# tile 0 loads via HWDGE (fast dispatch) while one mid store pays SWDGE.
TILES = (192, 1472, 1408, 1408, 1408, 1408, 640, 256)
LOAD_Q = ("sync",) + ("gpsimd",) * 7
STORE_Q = ("scalar", "sync", "scalar", "gpsimd",
           "scalar", "sync", "scalar", "sync")
WFIX = 64                       # host boundary fixup width

_COMPILED = {}
LAST_RESULTS = None             # BassKernelResults of the most recent run


def _coeffs(r):
    s = np.sqrt((1.0 + 2.0 * r) ** 2 - 4.0 * r * r)
    mu = ((1.0 + 2.0 * r) - s) / (2.0 * r)
    inv_delta = 2.0 / ((1.0 + 2.0 * r) + s)   # 1/delta, delta = steady denom
    return float(mu), float(inv_delta)


def _patch_tail_drain():
    """This environment's walrus build rejects instructions carrying more than
    ~1 semaphore wait.  Tile's kernel-tail drain aggregates one wait per live
    proc (engines + 8 DMA-HW queues) onto a single SP drain; split the extras
    onto dedicated single-wait nops just after it (all before the end
    barriers, so semantics are unchanged)."""
    import concourse.tile as tile

    if getattr(tile.TileContext, "_ant_split_drain", False):
        return

    def _drain_and_barrier(self, tick_clock, wait_clock):
        from concourse.vector_clock import ScopedClock
        from concourse import mybir

        drain_inst = self.nc.sync.drain()
        wait_clock.add_sem_waits(
            drain_inst.ins, ScopedClock({None: tick_clock.global_clock})
        )
        si = drain_inst.ins.sync_info
        waits = list(si.on_wait) if si is not None and si.on_wait else []
        if len(waits) > 1:
            drain_inst.ins.sync_info = mybir.SyncInfo(
                on_wait=[waits[0]], on_update=list(si.on_update or []))
            for w in waits[1:]:
                nop = self.nc.sync.nop(nofuse=True)
                nop.ins.sync_info = mybir.SyncInfo(on_wait=[w], on_update=[])

        self.nc.all_engine_barrier()
        assert self.sems is not None
        popped = self.nc._tile_sem_poison_stack.pop()
        assert popped is self._sem_poison
        self.nc.clear_and_free_semaphores(list(self.sems.allocated().values()))
        self.nc.all_engine_barrier()

    tile.TileContext._drain_and_barrier = _drain_and_barrier
    tile.TileContext._ant_split_drain = True


def _build_bass(mu, inv_delta):
    import concourse.bass as bass
    import concourse.tile as tile
    from concourse import mybir

    _patch_tail_drain()
    nc = bass.Bass()
    f32 = mybir.dt.float32
    # col 0 holds mu (the scan's data0 must be an SBUF tensor; keeping it in
    # the same DMA as tile 0's data keeps every instruction at <=1 sem wait)
    din = nc.dram_tensor("din", (P, 1 + FPT + 2 * H), f32, kind="ExternalInput")
    dout = nc.dram_tensor("dout", (P, FPT), f32, kind="ExternalOutput")

    mult, add = mybir.AluOpType.mult, mybir.AluOpType.add

    with tile.TileContext(nc) as tc:
        with tc.tile_pool(name="pool", bufs=2) as pool:
            cmu = None
            off = 0
            for t, T in enumerate(TILES):
                W = T + 2 * H
                tin = pool.tile([P, W + 1], f32, tag=f"tin{t}", bufs=1,
                                name=f"tin{t}")
                le = getattr(nc, LOAD_Q[t])
                if t == 0:
                    le.dma_start(out=tin, in_=din[:, 0 : 1 + W])
                    cmu = tin[:, 0:1]
                else:
                    le.dma_start(
                        out=tin[:, 1 : 1 + W],
                        in_=din[:, 1 + off : 1 + off + W])
                data = tin[:, 1 : 1 + W]
                # forward scan: v_i = mu*v_{i-1} + C_i/delta  (1/delta is
                # folded into the host-side input prep, so the backward scan
                # directly produces x and ScalarE stays out of the pipeline)
                u = pool.tile([P, W], f32, tag=f"u{t}", bufs=1, name=f"u{t}")
                nc.vector.tensor_tensor_scan(
                    out=u, data0=cmu.to_broadcast((P, W)), data1=data,
                    initial=0.0, op0=mult, op1=add)
                # backward scan: x_i = mu*x_{i+1} + v_i   (reversed traversal)
                y = pool.tile([P, W], f32, tag=f"y{t}", bufs=1, name=f"y{t}")
                nc.vector.tensor_tensor_scan(
                    out=y[:, ::-1], data0=cmu.to_broadcast((P, W)),
                    data1=u[:, ::-1], initial=0.0, op0=mult, op1=add)
                getattr(nc, STORE_Q[t]).dma_start(
                    out=dout[:, off : off + T], in_=y[:, H : H + T])
                off += T
    return nc


def _get_bass(mu, scale):
    key = (round(mu, 12), round(scale, 12))
    if key not in _COMPILED:
        _COMPILED[key] = _build_bass(mu, scale)
    return _COMPILED[key]


def _host_solve(C, mu, inv_delta):
    """Exact steady-state solve on host (float64), fully vectorized: the grid
    is viewed as 8192 chunks of 1024 whose recurrences run in lockstep; each
    chunk is seeded with the closed-form steady state of its left/right
    neighbour region (exact for the fixed-point recurrence)."""
    NCH, L = 8192, NX // 8192
    muL = mu ** L
    c2 = (C.astype(np.float64) * inv_delta).reshape(NCH, L)
    # local (zero-seeded) chunk sums in lockstep, then exact cross-chunk
    # carries V_k = local_k + mu^L * V_{k-1} via a small sequential pass
    s = np.zeros(NCH)
    for j in range(L):
        s = mu * s + c2[:, j]
    v_in = np.zeros(NCH)
    acc = 0.0
    for k in range(1, NCH):
        acc = s[k - 1] + muL * acc
        v_in[k] = acc
    v = np.zeros((NCH, L))
    s = v_in
    for j in range(L):
        s = mu * s + c2[:, j]
        v[:, j] = s
    s = np.zeros(NCH)
    for j in range(L - 1, -1, -1):
        s = mu * s + v[:, j]
    y_in = np.zeros(NCH)
    acc = 0.0
    for k in range(NCH - 2, -1, -1):
        acc = s[k + 1] + muL * acc
        y_in[k] = acc
    y = np.zeros((NCH, L))
    s = y_in
    for j in range(L - 1, -1, -1):
        s = mu * s + v[:, j]
        y[:, j] = s
    return y.reshape(-1).astype(np.float32)


def _thomas_f64(a, b, c, d):
    n = len(d)
    cp = np.zeros(n)
    dp = np.zeros(n)
    cp[0] = c[0] / b[0]
    dp[0] = d[0] / b[0]
    for i in range(1, n):
        den = b[i] - a[i] * cp[i - 1]
        cp[i] = c[i] / den
        dp[i] = (d[i] - a[i] * dp[i - 1]) / den
    x = np.zeros(n)
    x[-1] = dp[-1]
    for i in range(n - 2, -1, -1):
        x[i] = dp[i] - cp[i] * x[i + 1]
    return x


def _fix_boundaries(out, C, r, C_surf, C_bulk):
    """Overwrite the first/last WFIX entries with an exact float64 Thomas solve
    on a window, using the (interior-accurate) device value at the window's
    interior edge as far-field boundary condition."""
    n = WFIX + 1
    a = np.full(n, -r); b = np.full(n, 1.0 + 2.0 * r); c = np.full(n, -r)
    # left end: rows 0..WFIX, BCs x[0] = C_surf, x[WFIX] = out[WFIX]
    d = C[:n].astype(np.float64).copy()
    a[0] = 0.0; b[0] = 1.0; c[0] = 0.0; d[0] = C_surf
    a[-1] = 0.0; b[-1] = 1.0; c[-1] = 0.0; d[-1] = float(out[WFIX])
    out[:WFIX] = _thomas_f64(a, b, c, d)[:WFIX].astype(np.float32)
    # right end: rows nx-1-WFIX..nx-1, BCs x[left] = out[nx-1-WFIX], x[-1] = C_bulk
    a = np.full(n, -r); b = np.full(n, 1.0 + 2.0 * r); c = np.full(n, -r)
    d = C[-n:].astype(np.float64).copy()
    a[0] = 0.0; b[0] = 1.0; c[0] = 0.0; d[0] = float(out[len(out) - 1 - WFIX])
    a[-1] = 0.0; b[-1] = 1.0; c[-1] = 0.0; d[-1] = C_bulk
    out[len(out) - WFIX:] = _thomas_f64(a, b, c, d)[1:].astype(np.float32)


def kernel(**inputs):
    global LAST_RESULTS
    from concourse.bass_utils import run_bass_kernel_spmd

    C = np.asarray(inputs["C"], dtype=np.float32).reshape(-1)
    assert C.shape[0] == NX, f"expected {NX} grid points, got {C.shape}"
    dt = float(np.asarray(inputs["dt"]))
    C_surf = float(np.asarray(inputs["C_surf"]))
    C_bulk = float(np.asarray(inputs["C_bulk"]))
    D = float(np.asarray(inputs["D"]))
    dx = float(np.asarray(inputs["dx"]))

    r = D * dt / (dx * dx)
    if not np.isfinite(r) or r < 1e-12:
        out = C.copy()
        out[0] = np.float32(C_surf)
        out[-1] = np.float32(C_bulk)
        return out

    mu, inv_delta = _coeffs(r)
    if mu ** (H + 1) > 1e-8:
        # r large enough that the recurrence memory exceeds the baked-in
        # 8-element halo (needs r >~ 45; setup_inputs uses r = 0.1) — fall
        # back to an exact host solve rather than return degraded accuracy
        out = _host_solve(C, mu, inv_delta)
        _fix_boundaries(out, C, r, C_surf, C_bulk)
        return out
    nc = _get_bass(mu, inv_delta)

    # host-side sharding with halos (kernel reads C/delta; Dirichlet rows are
    # fixed up on host afterwards); col 0 of each per-core array carries mu
    Cp = np.zeros(NX + 2 * H, np.float32)
    np.multiply(C, np.float32(inv_delta), out=Cp[H : H + NX])
    in_maps = []
    for m in range(NCORES):
        w = Cp[m * SHARD : m * SHARD + SHARD + 2 * H]
        arr = np.empty((P, 1 + FPT + 2 * H), np.float32)
        arr[:, 0] = np.float32(mu)
        arr[:, 1:] = np.lib.stride_tricks.as_strided(
            w, shape=(P, FPT + 2 * H), strides=(FPT * 4, 4))
        in_maps.append({"din": arr})

    trace = os.environ.get("KBENCH_TRACE", "0") == "1"
    res = run_bass_kernel_spmd(
        nc, in_maps, core_ids=list(range(NCORES)), trace=trace)
    LAST_RESULTS = res

    out = np.empty(NX, np.float32)
    for m in range(NCORES):
        out[m * SHARD : (m + 1) * SHARD] = res.results[m]["dout"].reshape(-1)

    _fix_boundaries(out, C, r, C_surf, C_bulk)
    return out


# revision 28
# speedup vs baseline: 1.0022x; 1.0022x over previous
"""Backward-Euler 1D implicit diffusion step (tridiagonal solve) on 8 TRN2 cores.

Math: the system (I - dt*D*Lap) x = C has constant-coefficient tridiagonal
bands (-r, 1+2r, -r) away from the two Dirichlet rows, so away from the ends
the Thomas algorithm's elimination coefficients sit at their fixed point:
with s = sqrt((1+2r)^2 - 4r^2), mu = ((1+2r) - s)/(2r), delta = ((1+2r)+s)/2,
the solve is exactly
    v_i = mu*v_{i-1} + C_i      (forward first-order recurrence)
    y_i = mu*y_{i+1} + v_i      (backward first-order recurrence)
    x_i = y_i / delta
For r = 0.1, mu ~= 0.0839: the recurrences forget their initial state at
3e-10 relative within 8 elements, which makes the solve local — chunks can be
cut anywhere given an 8-element halo.  Both recurrences map 1:1 onto the DVE
``tensor_tensor_scan`` instruction ((mult, add) per-partition prefix scan;
the backward one runs on negative-stride APs), and the 1/delta scale is
folded into the host-side input prep, so the whole solve is 2 DVE scans per
tile.  The few boundary-influenced rows at each end (where the Thomas
coefficients have not yet converged to the fixed point and the Dirichlet rows
replace C) are fixed up on host with an exact float64 Thomas solve on a small
window.

Sharding: grid split into 8 contiguous shards, one per NeuronCore; halos are
materialized on host, so cores are fully independent (no collectives).
Per-core layout: partition p owns the contiguous sub-chunk
[p*8192, p*8192+8192) of the shard, stored with an 8-element halo per side as
a (128, 8209) DRAM tensor (col 0 carries mu — the scan's data0 operand must
live in SBUF, and folding it into tile 0's load keeps every instruction at
the <=1 semaphore wait this compiler build tolerates), processed as 8
pipelined tiles.  Loads ride SWDGE (gpsimd queue, otherwise
idle) so the HWDGE ring only carries stores — ring-slot waits would otherwise
stack a second wait onto the DMAs.  Tile sizes taper at both ends so the DVE
pipeline starts early and drains quickly."""

import os
import sys

import numpy as np

for _p in ("/opt/trn_rl_repo", "/root/.axon_site/_ro/trn_rl_repo"):
    if os.path.isdir(_p) and _p not in sys.path:
        sys.path.insert(0, _p)

NX = 8388608
NCORES = 8
P = 128
SHARD = NX // NCORES            # 1048576 per core
FPT = SHARD // P                # 8192 per partition
H = 8                           # halo (FIR half-width)
# per-core tile schedule: small head tile starts the DVE pipeline early,
# small tail tiles shorten the drain (store dispatch cost scales with width,
# so the last stores must be cheap); found by model search.  8 HWDGE DMAs max
# (the 8 DMA-HW ring procs each tolerate one without a ring-slot wait), so
# tile 0 loads via HWDGE (fast dispatch) while one mid store pays SWDGE.
TILES = (128, 1472, 1280, 1536, 1472, 1408, 576, 320)
LOAD_Q = ("sync",) + ("gpsimd",) * 7
STORE_Q = ("scalar", "sync", "scalar", "gpsimd",
           "scalar", "sync", "scalar", "sync")
WFIX = 64                       # host boundary fixup width

_COMPILED = {}
LAST_RESULTS = None             # BassKernelResults of the most recent run


def _coeffs(r):
    s = np.sqrt((1.0 + 2.0 * r) ** 2 - 4.0 * r * r)
    mu = ((1.0 + 2.0 * r) - s) / (2.0 * r)
    inv_delta = 2.0 / ((1.0 + 2.0 * r) + s)   # 1/delta, delta = steady denom
    return float(mu), float(inv_delta)


def _patch_tail_drain():
    """This environment's walrus build rejects instructions carrying more than
    ~1 semaphore wait.  Tile's kernel-tail drain aggregates one wait per live
    proc (engines + 8 DMA-HW queues) onto a single SP drain; split the extras
    onto dedicated single-wait nops just after it (all before the end
    barriers, so semantics are unchanged)."""
    import concourse.tile as tile

    if getattr(tile.TileContext, "_ant_split_drain", False):
        return

    def _drain_and_barrier(self, tick_clock, wait_clock):
        from concourse.vector_clock import ScopedClock
        from concourse import mybir

        drain_inst = self.nc.sync.drain()
        wait_clock.add_sem_waits(
            drain_inst.ins, ScopedClock({None: tick_clock.global_clock})
        )
        si = drain_inst.ins.sync_info
        waits = list(si.on_wait) if si is not None and si.on_wait else []
        if len(waits) > 1:
            drain_inst.ins.sync_info = mybir.SyncInfo(
                on_wait=[waits[0]], on_update=list(si.on_update or []))
            for w in waits[1:]:
                nop = self.nc.sync.nop(nofuse=True)
                nop.ins.sync_info = mybir.SyncInfo(on_wait=[w], on_update=[])

        self.nc.all_engine_barrier()
        assert self.sems is not None
        popped = self.nc._tile_sem_poison_stack.pop()
        assert popped is self._sem_poison
        self.nc.clear_and_free_semaphores(list(self.sems.allocated().values()))
        self.nc.all_engine_barrier()

    tile.TileContext._drain_and_barrier = _drain_and_barrier
    tile.TileContext._ant_split_drain = True


def _build_bass(mu, inv_delta):
    import concourse.bass as bass
    import concourse.tile as tile
    from concourse import mybir

    _patch_tail_drain()
    nc = bass.Bass()
    f32 = mybir.dt.float32
    # col 0 holds mu (the scan's data0 must be an SBUF tensor; keeping it in
    # the same DMA as tile 0's data keeps every instruction at <=1 sem wait)
    din = nc.dram_tensor("din", (P, 1 + FPT + 2 * H), f32, kind="ExternalInput")
    dout = nc.dram_tensor("dout", (P, FPT), f32, kind="ExternalOutput")

    mult, add = mybir.AluOpType.mult, mybir.AluOpType.add

    with tile.TileContext(nc) as tc:
        with tc.tile_pool(name="pool", bufs=2) as pool:
            cmu = None
            off = 0
            for t, T in enumerate(TILES):
                W = T + 2 * H
                tin = pool.tile([P, W + 1], f32, tag=f"tin{t}", bufs=1,
                                name=f"tin{t}")
                le = getattr(nc, LOAD_Q[t])
                if t == 0:
                    le.dma_start(out=tin, in_=din[:, 0 : 1 + W])
                    cmu = tin[:, 0:1]
                else:
                    le.dma_start(
                        out=tin[:, 1 : 1 + W],
                        in_=din[:, 1 + off : 1 + off + W])
                data = tin[:, 1 : 1 + W]
                # forward scan: v_i = mu*v_{i-1} + C_i/delta  (1/delta is
                # folded into the host-side input prep, so the backward scan
                # directly produces x and ScalarE stays out of the pipeline)
                u = pool.tile([P, W], f32, tag=f"u{t}", bufs=1, name=f"u{t}")
                nc.vector.tensor_tensor_scan(
                    out=u, data0=cmu.to_broadcast((P, W)), data1=data,
                    initial=0.0, op0=mult, op1=add)
                # backward scan: x_i = mu*x_{i+1} + v_i   (reversed traversal)
                y = pool.tile([P, W], f32, tag=f"y{t}", bufs=1, name=f"y{t}")
                nc.vector.tensor_tensor_scan(
                    out=y[:, ::-1], data0=cmu.to_broadcast((P, W)),
                    data1=u[:, ::-1], initial=0.0, op0=mult, op1=add)
                getattr(nc, STORE_Q[t]).dma_start(
                    out=dout[:, off : off + T], in_=y[:, H : H + T])
                off += T
    return nc


def _get_bass(mu, scale):
    key = (round(mu, 12), round(scale, 12))
    if key not in _COMPILED:
        _COMPILED[key] = _build_bass(mu, scale)
    return _COMPILED[key]


def _host_solve(C, mu, inv_delta):
    """Exact steady-state solve on host (float64), fully vectorized: the grid
    is viewed as 8192 chunks of 1024 whose recurrences run in lockstep; each
    chunk is seeded with the closed-form steady state of its left/right
    neighbour region (exact for the fixed-point recurrence)."""
    NCH, L = 8192, NX // 8192
    muL = mu ** L
    c2 = (C.astype(np.float64) * inv_delta).reshape(NCH, L)
    # local (zero-seeded) chunk sums in lockstep, then exact cross-chunk
    # carries V_k = local_k + mu^L * V_{k-1} via a small sequential pass
    s = np.zeros(NCH)
    for j in range(L):
        s = mu * s + c2[:, j]
    v_in = np.zeros(NCH)
    acc = 0.0
    for k in range(1, NCH):
        acc = s[k - 1] + muL * acc
        v_in[k] = acc
    v = np.zeros((NCH, L))
    s = v_in
    for j in range(L):
        s = mu * s + c2[:, j]
        v[:, j] = s
    s = np.zeros(NCH)
    for j in range(L - 1, -1, -1):
        s = mu * s + v[:, j]
    y_in = np.zeros(NCH)
    acc = 0.0
    for k in range(NCH - 2, -1, -1):
        acc = s[k + 1] + muL * acc
        y_in[k] = acc
    y = np.zeros((NCH, L))
    s = y_in
    for j in range(L - 1, -1, -1):
        s = mu * s + v[:, j]
        y[:, j] = s
    return y.reshape(-1).astype(np.float32)


def _thomas_f64(a, b, c, d):
    n = len(d)
    cp = np.zeros(n)
    dp = np.zeros(n)
    cp[0] = c[0] / b[0]
    dp[0] = d[0] / b[0]
    for i in range(1, n):
        den = b[i] - a[i] * cp[i - 1]
        cp[i] = c[i] / den
        dp[i] = (d[i] - a[i] * dp[i - 1]) / den
    x = np.zeros(n)
    x[-1] = dp[-1]
    for i in range(n - 2, -1, -1):
        x[i] = dp[i] - cp[i] * x[i + 1]
    return x


def _fix_boundaries(out, C, r, C_surf, C_bulk):
    """Overwrite the first/last WFIX entries with an exact float64 Thomas solve
    on a window, using the (interior-accurate) device value at the window's
    interior edge as far-field boundary condition."""
    n = WFIX + 1
    a = np.full(n, -r); b = np.full(n, 1.0 + 2.0 * r); c = np.full(n, -r)
    # left end: rows 0..WFIX, BCs x[0] = C_surf, x[WFIX] = out[WFIX]
    d = C[:n].astype(np.float64).copy()
    a[0] = 0.0; b[0] = 1.0; c[0] = 0.0; d[0] = C_surf
    a[-1] = 0.0; b[-1] = 1.0; c[-1] = 0.0; d[-1] = float(out[WFIX])
    out[:WFIX] = _thomas_f64(a, b, c, d)[:WFIX].astype(np.float32)
    # right end: rows nx-1-WFIX..nx-1, BCs x[left] = out[nx-1-WFIX], x[-1] = C_bulk
    a = np.full(n, -r); b = np.full(n, 1.0 + 2.0 * r); c = np.full(n, -r)
    d = C[-n:].astype(np.float64).copy()
    a[0] = 0.0; b[0] = 1.0; c[0] = 0.0; d[0] = float(out[len(out) - 1 - WFIX])
    a[-1] = 0.0; b[-1] = 1.0; c[-1] = 0.0; d[-1] = C_bulk
    out[len(out) - WFIX:] = _thomas_f64(a, b, c, d)[1:].astype(np.float32)


def kernel(**inputs):
    global LAST_RESULTS
    from concourse.bass_utils import run_bass_kernel_spmd

    C = np.asarray(inputs["C"], dtype=np.float32).reshape(-1)
    assert C.shape[0] == NX, f"expected {NX} grid points, got {C.shape}"
    dt = float(np.asarray(inputs["dt"]))
    C_surf = float(np.asarray(inputs["C_surf"]))
    C_bulk = float(np.asarray(inputs["C_bulk"]))
    D = float(np.asarray(inputs["D"]))
    dx = float(np.asarray(inputs["dx"]))

    r = D * dt / (dx * dx)
    if not np.isfinite(r) or r < 1e-12:
        out = C.copy()
        out[0] = np.float32(C_surf)
        out[-1] = np.float32(C_bulk)
        return out

    mu, inv_delta = _coeffs(r)
    if mu ** (H + 1) > 1e-8:
        # r large enough that the recurrence memory exceeds the baked-in
        # 8-element halo (needs r >~ 45; setup_inputs uses r = 0.1) — fall
        # back to an exact host solve rather than return degraded accuracy
        out = _host_solve(C, mu, inv_delta)
        _fix_boundaries(out, C, r, C_surf, C_bulk)
        return out
    nc = _get_bass(mu, inv_delta)

    # host-side sharding with halos (kernel reads C/delta; Dirichlet rows are
    # fixed up on host afterwards); col 0 of each per-core array carries mu
    Cp = np.zeros(NX + 2 * H, np.float32)
    np.multiply(C, np.float32(inv_delta), out=Cp[H : H + NX])
    in_maps = []
    for m in range(NCORES):
        w = Cp[m * SHARD : m * SHARD + SHARD + 2 * H]
        arr = np.empty((P, 1 + FPT + 2 * H), np.float32)
        arr[:, 0] = np.float32(mu)
        arr[:, 1:] = np.lib.stride_tricks.as_strided(
            w, shape=(P, FPT + 2 * H), strides=(FPT * 4, 4))
        in_maps.append({"din": arr})

    trace = os.environ.get("KBENCH_TRACE", "0") == "1"
    res = run_bass_kernel_spmd(
        nc, in_maps, core_ids=list(range(NCORES)), trace=trace)
    LAST_RESULTS = res

    out = np.empty(NX, np.float32)
    for m in range(NCORES):
        out[m * SHARD : (m + 1) * SHARD] = res.results[m]["dout"].reshape(-1)

    _fix_boundaries(out, C, r, C_surf, C_bulk)
    return out
